# revision 19
# baseline (speedup 1.0000x reference)
"""Trainium2 Bass kernel for nn_NodeGenerator (GNN message passing).

Strategy (8 NeuronCores, SPMD, no collectives):
  - Only candidate nodes (softmax class-0 > 0.5 and deg > 0) produce
    nonzero output rows. Candidates are compacted per core (~1550 of
    12500), so the neighbor-mean and the MLP run only on them.
  - The full node-feature table [100000, 64] f32 sits once in each
    core's HBM; dma_gather reads it through 4 quartile row-views so
    int16 indices stay in range.
  - Directed candidate edges sorted by (quartile, u-window); 128-edge
    tiles padded per (window, quartile) segment. Gathers run 4096
    indices per call (single_packet=False), 4 calls per quartile, so
    the Q7 per-call overhead nearly vanishes; per-core valid counts
    are reg-loaded so trailing pads emit no descriptors.
  - Per tile: a one-hot matrix S (iota vs u-local compare, fp16, DVE)
    and an accumulating fp16 PE matmul build neighbor sums. Each
    (window, quartile) run accumulates contiguously in one of 4
    rotating PSUM banks (a start=True matmul zeroes its whole 2KB
    zero-region), then is merged into an SBUF accumulator.
  - Neighbor mean + MLP run feature-major on 512-column chunks,
    interleaved into the call stream as soon as their windows merge.
  - Host assembles: device rows scatter back to candidate ids,
    non-candidates stay zero.
"""

import numpy as np

N = 100000
D = 64
CORES = 8
NPC = N // CORES
VQ = 25000
NQ = 4
CALL_TILES = 32         # 4096 indices per dma_gather (single_packet=False)


def _host_prep(node_features, node_operations, edge_index):
    fp16 = np.float16
    X = np.ascontiguousarray(np.asarray(node_features, dtype=np.float32))
    ops = np.asarray(node_operations, dtype=np.float32)
    ei = np.asarray(edge_index, dtype=np.int64)
    U = np.concatenate([ei[0], ei[1]])
    V = np.concatenate([ei[1], ei[0]])

    deg = np.bincount(U, minlength=N).astype(np.int64)
    o = ops.astype(np.float64)
    e = np.exp(o - o.max(axis=1, keepdims=True))
    p0 = e[:, 0] / e.sum(axis=1)
    mask = (p0 > 0.5) & (deg > 0)
    recip = (1.0 / np.maximum(deg, 1.0)).astype(np.float32)

    # compact candidates per core
    rank = np.full(N, -1, np.int64)
    cand_ids = []
    for c in range(CORES):
        ids = np.where(mask[c * NPC:(c + 1) * NPC])[0] + c * NPC
        rank[ids] = np.arange(len(ids))
        cand_ids.append(ids)
    cmax = max(len(i) for i in cand_ids)
    NW = -(-cmax // 128)            # candidate windows per core
    CP = NW * 128                   # padded candidate columns

    keep = mask[U]
    Uk, Vk = U[keep], V[keep]
    ck = Uk // NPC
    r = rank[Uk]
    w = r >> 7
    ul = (r & 127).astype(np.float32)
    q = Vk // VQ
    vloc = (Vk - q * VQ).astype(np.int16)

    # per-(quartile, window) tile caps shared across cores (SPMD)
    cnt = np.zeros((CORES, NQ, NW), np.int64)
    np.add.at(cnt, (ck, q, w), 1)
    cap = cnt.max(axis=0)                         # [NQ, NW]
    tqw = -(-cap // 128)                          # tiles per (q, w)
    tq = tqw.sum(axis=1)                          # tiles per quartile
    TA = int(tq.sum())
    sbase = np.zeros((NQ, NW), np.int64)          # global tile base
    np.cumsum(tqw.reshape(-1)[:-1], out=sbase.reshape(-1)[1:])
    qtb = np.zeros(NQ + 1, np.int64)
    np.cumsum(tq, out=qtb[1:])

    # slot assignment: edge -> global slot, contiguous per (core, q, w)
    gkey = (ck * NQ + q) * NW + w
    order = np.argsort(gkey, kind="stable")
    gk = gkey[order]
    ngroups = CORES * NQ * NW
    counts = np.bincount(gk, minlength=ngroups)
    starts = np.zeros(ngroups + 1, np.int64)
    np.cumsum(counts, out=starts[1:])
    within = np.arange(len(gk), dtype=np.int64) - starts[gk]
    qs = (gk // NW) % NQ
    ws = gk % NW
    cs_ = gk // (NQ * NW)
    pos = sbase[qs, ws] * 128 + within

    idxbuf = np.zeros((CORES, TA * 128), np.int16)      # interior pad: row 0
    ulbuf = np.full((CORES, TA * 128), -1.0, np.float32)
    idxbuf[cs_, pos] = vloc[order]
    ulbuf[cs_, pos] = ul[order]

    # per-core last real slot within each quartile (for trailing -1 pads)
    lastslot = np.full((CORES, NQ), -1, np.int64)       # inclusive
    np.maximum.at(lastslot, (cs_, qs), pos)

    # gather calls: per quartile, chunks of CALL_TILES tiles
    calls = []      # (q, tile_base, ntiles)
    for qq in range(NQ):
        for ts in range(0, int(tq[qq]), CALL_TILES):
            nt = min(CALL_TILES, int(tq[qq]) - ts)
            calls.append((qq, int(qtb[qq]) + ts, nt))
    # split the stream-final call so the last transfer tail is short
    if calls[-1][2] > 8:
        qq, tg, nt = calls[-1]
        calls[-1] = (qq, tg, nt - 8)
        calls.append((qq, tg + nt - 8, 8))
    NCALLS = len(calls)
    nvalid = np.zeros((CORES, NCALLS), np.int32)
    for k, (qq, tg, nt) in enumerate(calls):
        base = tg * 128
        v = np.clip(lastslot[:, qq] + 1 - base, 0, nt * 128)
        fake = v == 0
        v[fake] = 1                 # idxbuf already 0 there (valid row 0)
        nvalid[:, k] = v
        # slots beyond each core's valid count in this call: -1 (no desc)
        for c in range(CORES):
            idxbuf[c, base + int(v[c]):base + nt * 128] = -1

    # gather index stream: [CORES, 128, TA*8] (16-wrap, replicated x8)
    idx16 = idxbuf.reshape(CORES, TA * 8, 16).transpose(0, 2, 1)
    idx16 = np.ascontiguousarray(np.tile(idx16, (1, 8, 1)))
    # u-local planes: [CORES, 128(slot), TA]
    ulp = np.ascontiguousarray(
        ulbuf.reshape(CORES, TA, 128).transpose(0, 2, 1)).astype(fp16)

    # matmul runs: one per (q, w) with tiles, in global tile order
    runs = []
    for qq in range(NQ):
        for ww in range(NW):
            if tqw[qq, ww] == 0:
                continue
            t0 = int(sbase[qq, ww])
            runs.append(dict(
                idx=len(runs), t0=t0, t1=t0 + int(tqw[qq, ww]) - 1,
                win=ww, first=all(tqw[q2, ww] == 0 for q2 in range(qq))))
    run_of_tile = {}
    for rr in runs:
        for t in range(rr["t0"], rr["t1"] + 1):
            run_of_tile[t] = rr
    last_tile = np.full(NW, -1, np.int64)
    for rr in runs:
        last_tile[rr["win"]] = max(last_tile[rr["win"]], rr["t1"])
    NCH = -(-NW // 4)
    chunk_last = np.zeros(NCH, np.int64)
    for p in range(NCH):
        chunk_last[p] = max(last_tile[4 * p:min(4 * p + 4, NW)])

    # dense per-candidate inputs, feature-major, padded to CP columns
    xt = np.zeros((CORES, D, CP), fp16)
    rec = np.zeros((CORES, D, CP), np.float32)
    for c in range(CORES):
        k = len(cand_ids[c])
        xt[c, :, :k] = X[cand_ids[c]].T.astype(fp16)
        rec[c, :, :k] = np.broadcast_to(recip[cand_ids[c]], (D, k))

    return dict(X=X, NW=NW, CP=CP, TA=TA, calls=calls, nvalid=nvalid,
                runs=runs, run_of_tile=run_of_tile, chunk_last=chunk_last,
                NCH=NCH, idx16=idx16, ulp=ulp, xt=xt, rec=rec,
                cand_ids=cand_ids)


def _build(prep):
    from concourse import bacc, mybir, tile
    f32 = mybir.dt.float32
    fp16 = mybir.dt.float16
    i16 = mybir.dt.int16
    i32 = mybir.dt.int32
    AF = mybir.ActivationFunctionType
    ALU = mybir.AluOpType

    NW, CP, TA = prep["NW"], prep["CP"], prep["TA"]
    calls, NCH = prep["calls"], prep["NCH"]
    run_of_tile, chunk_last = prep["run_of_tile"], prep["chunk_last"]
    NCALLS = len(calls)

    nc = bacc.Bacc("TRN2", debug=False, num_swdge_queues=4)

    def din(name, shape, dt=f32):
        return nc.dram_tensor(name, shape, dt, kind="ExternalInput")

    xallh = din("xall", [N, D])
    idxh = din("idx", [128, TA * 8], i16)
    ulh = din("ul", [128, TA], fp16)
    nvh = din("nv", [1, NCALLS], i32)
    xth = din("xt", [D, CP], fp16)
    rech = din("rec", [D, CP])
    wpkh = din("wpk", [D, 356], fp16)
    w2h = din("w2", [128, D], fp16)
    bpkh = din("bpk", [128, 5])
    iotah = din("iot", [128, CALL_TILES, 128], fp16)
    o67h = nc.dram_tensor("o67", [67, CP], f32, kind="ExternalOutput")
    oph = nc.dram_tensor("op", [1, CP], f32, kind="ExternalOutput")

    with tile.TileContext(nc) as tc:
        with (
            tc.tile_pool(name="gconst", bufs=1) as gcpool,
            tc.tile_pool(name="const", bufs=1) as cpool,
            tc.tile_pool(name="xg", bufs=6) as gpool,
            tc.tile_pool(name="smat", bufs=2) as spool,
            tc.tile_pool(name="mlp", bufs=2) as mpool,
            tc.tile_pool(name="pseg", bufs=1, space="PSUM") as psseg,
            tc.tile_pool(name="pmlp", bufs=2, space="PSUM") as psmlp,
        ):
            def load_const(h, shape, dt=f32, eng=None, pool=None):
                nm = f"c_{h.name}"
                t = (pool or cpool).tile(shape, dt, name=nm, tag=nm)
                (eng or nc.sync).dma_start(t[:], h[:])
                return t

            # gather-critical consts first, in their own pool
            nv_t = load_const(nvh, [1, NCALLS], i32, pool=gcpool)
            idx_t = load_const(idxh, [128, TA * 8], i16, pool=gcpool)
            ul_t = load_const(ulh, [128, TA], fp16, nc.scalar, pool=gcpool)
            iota = load_const(iotah, [128, CALL_TILES, 128], fp16, nc.scalar)
            xt_t = load_const(xth, [D, CP], fp16, nc.scalar)
            rec_t = load_const(rech, [D, CP], f32, nc.scalar)
            wpk_t = load_const(wpkh, [D, 356], fp16)
            w2_t = load_const(w2h, [128, D], fp16)
            bpk_t = load_const(bpkh, [128, 5])
            w1a_t = wpk_t[:, 0:128]
            w1b_t = wpk_t[:, 128:256]
            w3_t = wpk_t[:, 256:323]
            p1_t = wpk_t[:, 323:355]
            p2_t = wpk_t[:32, 355:356]
            b1_t = bpk_t[:, 0:1]
            b2_t = bpk_t[:D, 1:2]
            b3_t = bpk_t[:67, 2:3]
            pb1_t = bpk_t[:32, 3:4]
            pb2_t = bpk_t[:1, 4:5]

            xgb = cpool.tile([128, TA, D], fp16, name="xgb", tag="xgb")
            acc = cpool.tile([D, CP], f32, name="acc", tag="acc")
            # 4 rotating accumulators, each a full 2KB PSUM zero-region
            ps = [psseg.tile([D, 512], f32, name=f"ps{p}", tag=f"ps{p}")
                  for p in range(4)]
            nvreg = nc.alloc_register(mybir.EngineType.Pool, "nvreg")

            def emit_chunk(p):
                c0 = 512 * p
                cs = min(512, CP - c0)
                nmean = mpool.tile([D, 512], fp16, tag="nmean")
                nc.vector.tensor_tensor(
                    out=nmean[:, :cs], in0=acc[:, c0:c0 + cs],
                    in1=rec_t[:, c0:c0 + cs], op=ALU.mult)
                h1p = psmlp.tile([128, 512], f32, tag="big")
                nc.tensor.matmul(h1p[:, :cs], lhsT=w1a_t,
                                 rhs=xt_t[:, c0:c0 + cs],
                                 start=True, stop=False)
                nc.tensor.matmul(h1p[:, :cs], lhsT=w1b_t,
                                 rhs=nmean[:, :cs], start=False, stop=True)
                h1 = mpool.tile([128, 512], fp16, tag="h1")
                nc.scalar.activation(out=h1[:, :cs], in_=h1p[:, :cs],
                                     func=AF.Relu, bias=b1_t, scale=1.0)
                h2p = psmlp.tile([D, 512], f32, tag="big")
                nc.tensor.matmul(h2p[:, :cs], lhsT=w2_t[:], rhs=h1[:, :cs],
                                 start=True, stop=True)
                h2 = mpool.tile([D, 512], fp16, tag="h2")
                nc.scalar.activation(out=h2[:, :cs], in_=h2p[:, :cs],
                                     func=AF.Relu, bias=b2_t, scale=1.0)
                gp = psmlp.tile([67, 512], f32, tag="small")
                nc.tensor.matmul(gp[:, :cs], lhsT=w3_t, rhs=h2[:, :cs],
                                 start=True, stop=True)
                g67 = mpool.tile([67, 512], f32, tag="g67")
                nc.scalar.activation(out=g67[:, :cs], in_=gp[:, :cs],
                                     func=AF.Identity, bias=b3_t,
                                     scale=1.0)
                g64b = mpool.tile([D, 512], fp16, tag="g64b")
                nc.scalar.copy(g64b[:, :cs], g67[:D, :cs])
                pp = psmlp.tile([32, 512], f32, tag="small")
                nc.tensor.matmul(pp[:, :cs], lhsT=p1_t, rhs=g64b[:, :cs],
                                 start=True, stop=True)
                pa = mpool.tile([32, 512], fp16, tag="pa")
                nc.scalar.activation(out=pa[:, :cs], in_=pp[:, :cs],
                                     func=AF.Relu, bias=pb1_t, scale=1.0)
                prp = psmlp.tile([1, 512], f32, tag="small")
                nc.tensor.matmul(prp[:, :cs], lhsT=p2_t, rhs=pa[:, :cs],
                                 start=True, stop=True)
                pr = mpool.tile([1, 512], f32, tag="pr")
                nc.scalar.activation(out=pr[:, :cs], in_=prp[:, :cs],
                                     func=AF.Sigmoid, bias=pb2_t,
                                     scale=1.0)
                nc.sync.dma_start(o67h[:, c0:c0 + cs], g67[:, :cs])
                nc.sync.dma_start(oph[:, c0:c0 + cs], pr[:, :cs])

            nvalid = prep["nvalid"]
            next_chunk = 0
            for k, (qq, tg, nt) in enumerate(calls):
                xg = gpool.tile([128, CALL_TILES, D], f32, tag="xg")
                uniform = (nvalid[:, k] == nvalid[0, k]).all()
                if uniform:
                    nvarg = int(nvalid[0, k])
                else:
                    nc.gpsimd.reg_load(nvreg, nv_t[0:1, k:k + 1])
                    nvarg = nvreg
                nc.gpsimd.dma_gather(
                    xg[:, :nt, :], xallh[qq * VQ:(qq + 1) * VQ, :],
                    idx_t[:, tg * 8:(tg + nt) * 8],
                    nt * 128, nvarg, D, queue_num=k % 4,
                    single_packet=False)
                nc.scalar.copy(xgb[:, tg:tg + nt, :], xg[:, :nt, :])
                S = spool.tile([128, CALL_TILES, 128], fp16, tag="S",
                               name=f"S{k}")
                nc.vector.tensor_tensor(
                    out=S[:, :nt, :], in0=iota[:, :nt, :],
                    in1=ul_t[:, tg:tg + nt].broadcast_to([128, nt, 128]),
                    op=ALU.is_equal)
                for t in range(tg, tg + nt):
                    rr = run_of_tile[t]
                    pz = ps[rr["idx"] % 4]
                    nc.tensor.matmul(
                        pz[:, :128],
                        lhsT=xgb[:, t, :], rhs=S[:, t - tg, :],
                        start=(t == rr["t0"]), stop=(t == rr["t1"]))
                    if t == rr["t1"]:
                        ww = rr["win"]
                        asl = acc[:, ww * 128:(ww + 1) * 128]
                        if rr["first"]:
                            nc.scalar.copy(asl, pz[:, :128])
                        else:
                            nc.vector.tensor_tensor(
                                out=asl, in0=asl, in1=pz[:, :128],
                                op=ALU.add)
                while (next_chunk < NCH
                       and chunk_last[next_chunk] < tg + nt):
                    emit_chunk(next_chunk)
                    next_chunk += 1
            while next_chunk < NCH:
                emit_chunk(next_chunk)
                next_chunk += 1

    nc.compile()
    return nc


def _in_maps(prep, W1, b1, W2, b2, W3, b3, P1, pb1, P2, pb2):
    fp16 = np.float16
    W1 = np.asarray(W1, np.float32)
    W3 = np.asarray(W3, np.float32)
    b3 = np.asarray(b3, np.float32)
    w3p = np.concatenate([W3[:, 3:], W3[:, :3]], axis=1)   # [feats|pos]
    b3p = np.concatenate([b3[3:], b3[:3]])
    wpk = np.zeros((D, 356), np.float32)
    wpk[:, 0:128] = W1[:D]
    wpk[:, 128:256] = W1[D:]
    wpk[:, 256:323] = w3p
    wpk[:, 323:355] = np.asarray(P1, np.float32)
    wpk[:32, 355:356] = np.asarray(P2, np.float32)
    bpk = np.zeros((128, 5), np.float32)
    bpk[:, 0] = np.asarray(b1, np.float32)
    bpk[:D, 1] = np.asarray(b2, np.float32)
    bpk[:67, 2] = b3p
    bpk[:32, 3] = np.asarray(pb1, np.float32)
    bpk[:1, 4] = np.asarray(pb2, np.float32)
    iot = np.broadcast_to(np.arange(128, dtype=np.float16),
                          (128, CALL_TILES, 128))
    shared = {
        "xall": prep["X"],
        "iot": np.ascontiguousarray(iot),
        "wpk": wpk.astype(fp16),
        "w2": np.asarray(W2, np.float32).astype(fp16),
        "bpk": bpk,
    }
    maps = []
    for c in range(CORES):
        m = dict(shared)
        m["idx"] = prep["idx16"][c]
        m["ul"] = prep["ulp"][c]
        m["nv"] = prep["nvalid"][c:c + 1]
        m["xt"] = prep["xt"][c]
        m["rec"] = prep["rec"][c]
        maps.append(m)
    return maps


def _assemble(results, prep):
    out = np.zeros((N, D + 4), np.float32)
    for c, r in enumerate(results):
        ids = prep["cand_ids"][c]
        k = len(ids)
        o67 = r["o67"][:, :k]
        out[ids, 0:3] = o67[D:D + 3].T
        out[ids, 3:3 + D] = o67[:D].T
        out[ids, 3 + D] = r["op"][0, :k]
    return out


def kernel(**inputs):
    from concourse.bass_utils import run_bass_kernel_spmd
    prep = _host_prep(inputs["node_features"], inputs["node_operations"],
                      inputs["edge_index"])
    nc = _build(prep)
    maps = _in_maps(prep, inputs["W1"], inputs["b1"], inputs["W2"],
                    inputs["b2"], inputs["W3"], inputs["b3"], inputs["P1"],
                    inputs["pb1"], inputs["P2"], inputs["pb2"])
    res = run_bass_kernel_spmd(nc, maps, core_ids=list(range(CORES)))
    return _assemble(res.results, prep)


# revision 22
# speedup vs baseline: 1.2179x; 1.2179x over previous
"""Trainium2 Bass kernel for nn_NodeGenerator (GNN message passing).

Strategy (8 NeuronCores, SPMD, no collectives):
  - Only candidate nodes (softmax class-0 > 0.5 and deg > 0) produce
    nonzero output rows. Candidates are compacted per core (~1550 of
    12500), so the neighbor-mean and the MLP run only on them.
  - The full node-feature table [100000, 64] f32 sits once in each
    core's HBM; dma_gather reads it through 4 quartile row-views so
    int16 indices stay in range.
  - Directed candidate edges sorted by (quartile, u-window) and packed
    contiguously per (core, quartile) — no per-window padding. 128-edge
    tiles may straddle windows; each (tile, window) intersection in ANY
    core becomes one matmul job (same program on all cores). Trailing
    slots per quartile hold -1 indices, which the DGE skips, so the Q7
    descriptor loop only pays for real edges (~49K/core, the hard wall
    at ~3.8ns/descriptor).
  - Gathers run 4096 indices per call (single_packet=False), ~4 calls
    per quartile; per-core valid counts are reg-loaded when they differ
    across cores.
  - Per job: a one-hot matrix S (iota vs u-local compare, fp16, DVE)
    and an accumulating fp16 PE matmul build neighbor sums. Each
    (quartile, window) group owns one of 4 rotating PSUM banks (a
    start=True matmul zeroes its whole 2KB zero-region) and is merged
    into an SBUF accumulator after its last job.
  - Neighbor mean + MLP run feature-major on 512-column chunks,
    interleaved into the call stream as soon as their windows merge.
  - Host assembles: device rows scatter back to candidate ids,
    non-candidates stay zero.
"""

import numpy as np

N = 100000
D = 64
CORES = 8
NPC = N // CORES
VQ = 25000
NQ = 4
CALL_TILES = 32         # 4096 indices per dma_gather (single_packet=False)
SBATCH = 8              # tiles per S-matrix is_equal sub-batch


def _host_prep(node_features, node_operations, edge_index):
    fp16 = np.float16
    X = np.ascontiguousarray(np.asarray(node_features, dtype=np.float32))
    ops = np.asarray(node_operations, dtype=np.float32)
    ei = np.asarray(edge_index, dtype=np.int64)
    U = np.concatenate([ei[0], ei[1]])
    V = np.concatenate([ei[1], ei[0]])

    deg = np.bincount(U, minlength=N).astype(np.int64)
    o = ops.astype(np.float64)
    e = np.exp(o - o.max(axis=1, keepdims=True))
    p0 = e[:, 0] / e.sum(axis=1)
    mask = (p0 > 0.5) & (deg > 0)
    recip = (1.0 / np.maximum(deg, 1.0)).astype(np.float32)

    # compact candidates per core
    rank = np.full(N, -1, np.int64)
    cand_ids = []
    for c in range(CORES):
        ids = np.where(mask[c * NPC:(c + 1) * NPC])[0] + c * NPC
        rank[ids] = np.arange(len(ids))
        cand_ids.append(ids)
    cmax = max(len(i) for i in cand_ids)
    NW = -(-cmax // 128)            # candidate windows per core
    CP = NW * 128                   # padded candidate columns

    keep = mask[U]
    Uk, Vk = U[keep], V[keep]
    ck = Uk // NPC
    r = rank[Uk]
    w = r >> 7
    ul = (r & 127).astype(np.float32)
    q = Vk // VQ
    vloc = (Vk - q * VQ).astype(np.int16)

    cnt = np.zeros((CORES, NQ, NW), np.int64)
    np.add.at(cnt, (ck, q, w), 1)
    cnt_cq = cnt.sum(axis=2)                      # [CORES, NQ]
    tq = -(-cnt_cq.max(axis=0) // 128)            # tiles per quartile
    TA = int(tq.sum())
    qtb = np.zeros(NQ + 1, np.int64)
    np.cumsum(tq, out=qtb[1:])
    cumw = np.zeros((CORES, NQ, NW), np.int64)    # excl cumsum over w
    np.cumsum(cnt[:, :, :-1], axis=2, out=cumw[:, :, 1:])

    # slot assignment: contiguous per (core, quartile), w-sorted
    gkey = (ck * NQ + q) * NW + w
    order = np.argsort(gkey, kind="stable")
    gk = gkey[order]
    ngroups = CORES * NQ * NW
    counts = np.bincount(gk, minlength=ngroups)
    starts = np.zeros(ngroups + 1, np.int64)
    np.cumsum(counts, out=starts[1:])
    within = np.arange(len(gk), dtype=np.int64) - starts[gk]
    qs = (gk // NW) % NQ
    ws = gk % NW
    cs_ = gk // (NQ * NW)
    pos = qtb[qs] * 128 + cumw[cs_, qs, ws] + within

    idxbuf = np.full((CORES, TA * 128), -1, np.int16)
    idxbuf[cs_, pos] = vloc[order]

    # union matmul jobs: (quartile, global tile, window)
    jobset = set()
    for c in range(CORES):
        for qq in range(NQ):
            for ww in range(NW):
                n_ = cnt[c, qq, ww]
                if n_ == 0:
                    continue
                lo = int(cumw[c, qq, ww]) // 128
                hi = int(cumw[c, qq, ww] + n_ - 1) // 128
                for t in range(lo, hi + 1):
                    jobset.add((qq, int(qtb[qq]) + t, ww))
    jobs = sorted(jobset)                          # (q, t, w) order
    J = len(jobs)
    jkeys = np.array([t * NW + ww for (_, t, ww) in jobs], np.int64)
    j_e = np.searchsorted(jkeys, (pos // 128) * NW + ws)

    ulj = np.full((CORES, 128, J), -1.0, np.float32)
    ulj[cs_, pos % 128, j_e] = ul[order]
    ulj = ulj.astype(fp16)

    # (q, w) groups: first/last job, copy-vs-add, window completion
    jinfo = []
    gidx = {}
    glast = {}
    for j, (qq, t, ww) in enumerate(jobs):
        g = (qq, ww)
        if g not in gidx:
            gidx[g] = (len(gidx), j)
        glast[g] = j
        jinfo.append(dict(t=t, win=ww, grp=gidx[g][0]))
    wlast = {}
    firstq = {}
    for (qq, ww), j in glast.items():
        wlast[ww] = max(wlast.get(ww, -1), j)
        firstq[ww] = min(firstq.get(ww, NQ), qq)
    for j, (qq, t, ww) in enumerate(jobs):
        g = (qq, ww)
        jinfo[j]["start"] = gidx[g][0] >= 0 and gidx[g][1] == j
        jinfo[j]["stop"] = glast[g] == j
        jinfo[j]["merge"] = jinfo[j]["stop"]
        jinfo[j]["copy"] = jinfo[j]["stop"] and qq == firstq[ww]
        jinfo[j]["windone"] = wlast[ww] == j

    # gather calls: per quartile, chunks of CALL_TILES tiles
    calls = []      # (q, tile_base, ntiles)
    for qq in range(NQ):
        for ts in range(0, int(tq[qq]), CALL_TILES):
            nt = min(CALL_TILES, int(tq[qq]) - ts)
            calls.append((qq, int(qtb[qq]) + ts, nt))
    # small first call (cold-Q7 penalty) and small final call (tail)
    if calls[0][2] > 8:
        qq, tg, nt = calls[0]
        calls[0] = (qq, tg, 8)
        calls.insert(1, (qq, tg + 8, nt - 8))
    if calls[-1][2] > 8:
        qq, tg, nt = calls[-1]
        calls[-1] = (qq, tg, nt - 8)
        calls.append((qq, tg + nt - 8, 8))
    NCALLS = len(calls)
    nvalid = np.zeros((CORES, NCALLS), np.int32)
    for k, (qq, tg, nt) in enumerate(calls):
        off = (tg - qtb[qq]) * 128
        v = np.clip(cnt_cq[:, qq] - off, 0, nt * 128).astype(np.int32)
        fake = v == 0
        if fake.any():
            idxbuf[fake, tg * 128] = 0            # 1 dummy desc
            v[fake] = 1
        nvalid[:, k] = v
    # jobs per call: contiguous [jlo, jhi)
    tstarts = np.array([t for (_, t, _) in jobs], np.int64)
    call_jobs = []
    for (qq, tg, nt) in calls:
        jlo = int(np.searchsorted(tstarts, tg, side="left"))
        jhi = int(np.searchsorted(tstarts, tg + nt, side="left"))
        call_jobs.append((jlo, jhi))

    # gather index stream: [CORES, 128, TA*8] (16-wrap, replicated x8)
    idx16 = idxbuf.reshape(CORES, TA * 8, 16).transpose(0, 2, 1)
    idx16 = np.ascontiguousarray(np.tile(idx16, (1, 8, 1)))

    NCH = -(-NW // 4)
    # dense per-candidate inputs, feature-major, padded to CP columns
    xt = np.zeros((CORES, D, CP), fp16)
    rec = np.zeros((CORES, D, CP), np.float32)
    for c in range(CORES):
        k = len(cand_ids[c])
        xt[c, :, :k] = X[cand_ids[c]].T.astype(fp16)
        rec[c, :, :k] = np.broadcast_to(recip[cand_ids[c]], (D, k))

    return dict(X=X, NW=NW, CP=CP, TA=TA, J=J, calls=calls,
                call_jobs=call_jobs, jinfo=jinfo, nvalid=nvalid,
                NCH=NCH, idx16=idx16, ulj=ulj, xt=xt, rec=rec,
                cand_ids=cand_ids)


def _build(prep):
    from concourse import bacc, mybir, tile
    f32 = mybir.dt.float32
    fp16 = mybir.dt.float16
    i16 = mybir.dt.int16
    i32 = mybir.dt.int32
    AF = mybir.ActivationFunctionType
    ALU = mybir.AluOpType

    NW, CP, TA, J = prep["NW"], prep["CP"], prep["TA"], prep["J"]
    calls, call_jobs = prep["calls"], prep["call_jobs"]
    jinfo, NCH = prep["jinfo"], prep["NCH"]
    nvalid = prep["nvalid"]
    NCALLS = len(calls)
    nt0 = calls[0][2]
    SW = max(hi - lo for lo, hi in call_jobs)   # max jobs per call

    nc = bacc.Bacc("TRN2", debug=False, num_swdge_queues=4)

    def din(name, shape, dt=f32):
        return nc.dram_tensor(name, shape, dt, kind="ExternalInput")

    xallh = din("xall", [N, D])
    idx0h = din("idx0", [128, nt0 * 8], i16)
    idxrh = din("idxr", [128, (TA - nt0) * 8], i16)
    ulh = din("ul", [128, J], fp16)
    nvh = din("nv", [1, NCALLS], i32)
    xth = din("xt", [D, CP], fp16)
    rech = din("rec", [D, CP])
    wpkh = din("wpk", [D, 356], fp16)
    w2h = din("w2", [128, D], fp16)
    bpkh = din("bpk", [128, 5])
    iotah = din("iot", [128, SBATCH, 128], fp16)
    o67h = nc.dram_tensor("o67", [67, CP], f32, kind="ExternalOutput")
    oph = nc.dram_tensor("op", [1, CP], f32, kind="ExternalOutput")

    with tile.TileContext(nc) as tc:
        with (
            tc.tile_pool(name="gconst", bufs=1) as gcpool,
            tc.tile_pool(name="const", bufs=1) as cpool,
            tc.tile_pool(name="xg", bufs=6) as gpool,
            tc.tile_pool(name="smat", bufs=2) as spool,
            tc.tile_pool(name="mlp", bufs=2) as mpool,
            tc.tile_pool(name="pseg", bufs=1, space="PSUM") as psseg,
            tc.tile_pool(name="pmlp", bufs=2, space="PSUM") as psmlp,
        ):
            def load_const(h, shape, dt=f32, eng=None, pool=None):
                nm = f"c_{h.name}"
                t = (pool or cpool).tile(shape, dt, name=nm, tag=nm)
                (eng or nc.sync).dma_start(t[:], h[:])
                return t

            # gather-critical consts first, small ones leading
            idx0_t = load_const(idx0h, [128, nt0 * 8], i16, pool=gcpool)
            iota = load_const(iotah, [128, SBATCH, 128], fp16, nc.scalar,
                              pool=gcpool)
            ul_t = load_const(ulh, [128, J], fp16, nc.scalar, pool=gcpool)
            nv_t = load_const(nvh, [1, NCALLS], i32, pool=gcpool)
            idxr_t = load_const(idxrh, [128, (TA - nt0) * 8], i16,
                                pool=gcpool)
            xt_t = load_const(xth, [D, CP], fp16, nc.scalar)
            rec_t = load_const(rech, [D, CP], f32, nc.scalar)
            wpk_t = load_const(wpkh, [D, 356], fp16)
            w2_t = load_const(w2h, [128, D], fp16)
            bpk_t = load_const(bpkh, [128, 5])
            w1a_t = wpk_t[:, 0:128]
            w1b_t = wpk_t[:, 128:256]
            w3_t = wpk_t[:, 256:323]
            p1_t = wpk_t[:, 323:355]
            p2_t = wpk_t[:32, 355:356]
            b1_t = bpk_t[:, 0:1]
            b2_t = bpk_t[:D, 1:2]
            b3_t = bpk_t[:67, 2:3]
            pb1_t = bpk_t[:32, 3:4]
            pb2_t = bpk_t[:1, 4:5]

            xgb = cpool.tile([128, TA, D], fp16, name="xgb", tag="xgb")
            acc = cpool.tile([D, CP], f32, name="acc", tag="acc")
            # 4 rotating accumulators, each a full 2KB PSUM zero-region
            ps = [psseg.tile([D, 512], f32, name=f"ps{p}", tag=f"ps{p}")
                  for p in range(4)]
            nvreg = nc.alloc_register(mybir.EngineType.Pool, "nvreg")

            def emit_chunk(p):
                c0 = 512 * p
                cs = min(512, CP - c0)
                nmean = mpool.tile([D, 512], fp16, tag="nmean")
                nc.vector.tensor_tensor(
                    out=nmean[:, :cs], in0=acc[:, c0:c0 + cs],
                    in1=rec_t[:, c0:c0 + cs], op=ALU.mult)
                h1p = psmlp.tile([128, 512], f32, tag="big")
                nc.tensor.matmul(h1p[:, :cs], lhsT=w1a_t,
                                 rhs=xt_t[:, c0:c0 + cs],
                                 start=True, stop=False)
                nc.tensor.matmul(h1p[:, :cs], lhsT=w1b_t,
                                 rhs=nmean[:, :cs], start=False, stop=True)
                h1 = mpool.tile([128, 512], fp16, tag="h1")
                nc.scalar.activation(out=h1[:, :cs], in_=h1p[:, :cs],
                                     func=AF.Relu, bias=b1_t, scale=1.0)
                h2p = psmlp.tile([D, 512], f32, tag="big")
                nc.tensor.matmul(h2p[:, :cs], lhsT=w2_t[:], rhs=h1[:, :cs],
                                 start=True, stop=True)
                h2 = mpool.tile([D, 512], fp16, tag="h2")
                nc.scalar.activation(out=h2[:, :cs], in_=h2p[:, :cs],
                                     func=AF.Relu, bias=b2_t, scale=1.0)
                gp = psmlp.tile([67, 512], f32, tag="small")
                nc.tensor.matmul(gp[:, :cs], lhsT=w3_t, rhs=h2[:, :cs],
                                 start=True, stop=True)
                g67 = mpool.tile([67, 512], f32, tag="g67")
                nc.scalar.activation(out=g67[:, :cs], in_=gp[:, :cs],
                                     func=AF.Identity, bias=b3_t,
                                     scale=1.0)
                g64b = mpool.tile([D, 512], fp16, tag="g64b")
                nc.scalar.copy(g64b[:, :cs], g67[:D, :cs])
                pp = psmlp.tile([32, 512], f32, tag="small")
                nc.tensor.matmul(pp[:, :cs], lhsT=p1_t, rhs=g64b[:, :cs],
                                 start=True, stop=True)
                pa = mpool.tile([32, 512], fp16, tag="pa")
                nc.scalar.activation(out=pa[:, :cs], in_=pp[:, :cs],
                                     func=AF.Relu, bias=pb1_t, scale=1.0)
                prp = psmlp.tile([1, 512], f32, tag="small")
                nc.tensor.matmul(prp[:, :cs], lhsT=p2_t, rhs=pa[:, :cs],
                                 start=True, stop=True)
                pr = mpool.tile([1, 512], f32, tag="pr")
                nc.scalar.activation(out=pr[:, :cs], in_=prp[:, :cs],
                                     func=AF.Sigmoid, bias=pb2_t,
                                     scale=1.0)
                nc.sync.dma_start(o67h[:, c0:c0 + cs], g67[:, :cs])
                nc.sync.dma_start(oph[:, c0:c0 + cs], pr[:, :cs])

            windone = [False] * NW
            next_chunk = 0
            for k, (qq, tg, nt) in enumerate(calls):
                xg = gpool.tile([128, CALL_TILES, D], f32, tag="xg")
                uniform = (nvalid[:, k] == nvalid[0, k]).all()
                if uniform:
                    nvarg = int(nvalid[0, k])
                else:
                    nc.gpsimd.reg_load(nvreg, nv_t[0:1, k:k + 1])
                    nvarg = nvreg
                if k == 0:
                    idxap = idx0_t[:, 0:nt * 8]
                else:
                    idxap = idxr_t[:, (tg - nt0) * 8:(tg + nt - nt0) * 8]
                nc.gpsimd.dma_gather(
                    xg[:, :nt, :], xallh[qq * VQ:(qq + 1) * VQ, :],
                    idxap, nt * 128, nvarg, D, queue_num=k % 4,
                    single_packet=False)
                nc.scalar.copy(xgb[:, tg:tg + nt, :], xg[:, :nt, :])
                jlo, jhi = call_jobs[k]
                nj = jhi - jlo
                S = spool.tile([128, SW, 128], fp16, tag="S", name=f"S{k}")
                for b0 in range(0, nj, SBATCH):
                    nb = min(SBATCH, nj - b0)
                    nc.vector.tensor_tensor(
                        out=S[:, b0:b0 + nb, :], in0=iota[:, :nb, :],
                        in1=ul_t[:, jlo + b0:jlo + b0 + nb]
                            .broadcast_to([128, nb, 128]),
                        op=ALU.is_equal)
                for j in range(jlo, jhi):
                    ji = jinfo[j]
                    t, ww = ji["t"], ji["win"]
                    pz = ps[ji["grp"] % 4]
                    nc.tensor.matmul(
                        pz[:, :128],
                        lhsT=xgb[:, t, :], rhs=S[:, j - jlo, :],
                        start=ji["start"], stop=ji["stop"])
                    if ji["merge"]:
                        asl = acc[:, ww * 128:(ww + 1) * 128]
                        if ji["copy"]:
                            nc.scalar.copy(asl, pz[:, :128])
                        else:
                            nc.vector.tensor_tensor(
                                out=asl, in0=asl, in1=pz[:, :128],
                                op=ALU.add)
                        if ji["windone"]:
                            windone[ww] = True
                    while (next_chunk < NCH
                           and all(windone[4 * next_chunk:
                                           min(4 * next_chunk + 4, NW)])):
                        emit_chunk(next_chunk)
                        next_chunk += 1
            while next_chunk < NCH:
                emit_chunk(next_chunk)
                next_chunk += 1

    nc.compile()
    return nc


def _in_maps(prep, W1, b1, W2, b2, W3, b3, P1, pb1, P2, pb2):
    fp16 = np.float16
    W1 = np.asarray(W1, np.float32)
    W3 = np.asarray(W3, np.float32)
    b3 = np.asarray(b3, np.float32)
    w3p = np.concatenate([W3[:, 3:], W3[:, :3]], axis=1)   # [feats|pos]
    b3p = np.concatenate([b3[3:], b3[:3]])
    wpk = np.zeros((D, 356), np.float32)
    wpk[:, 0:128] = W1[:D]
    wpk[:, 128:256] = W1[D:]
    wpk[:, 256:323] = w3p
    wpk[:, 323:355] = np.asarray(P1, np.float32)
    wpk[:32, 355:356] = np.asarray(P2, np.float32)
    bpk = np.zeros((128, 5), np.float32)
    bpk[:, 0] = np.asarray(b1, np.float32)
    bpk[:D, 1] = np.asarray(b2, np.float32)
    bpk[:67, 2] = b3p
    bpk[:32, 3] = np.asarray(pb1, np.float32)
    bpk[:1, 4] = np.asarray(pb2, np.float32)
    iot = np.broadcast_to(np.arange(128, dtype=np.float16),
                          (128, SBATCH, 128))
    nt0 = prep["calls"][0][2]
    shared = {
        "xall": prep["X"],
        "iot": np.ascontiguousarray(iot),
        "wpk": wpk.astype(fp16),
        "w2": np.asarray(W2, np.float32).astype(fp16),
        "bpk": bpk,
    }
    maps = []
    for c in range(CORES):
        m = dict(shared)
        m["idx0"] = np.ascontiguousarray(prep["idx16"][c][:, :nt0 * 8])
        m["idxr"] = np.ascontiguousarray(prep["idx16"][c][:, nt0 * 8:])
        m["ul"] = prep["ulj"][c]
        m["nv"] = prep["nvalid"][c:c + 1]
        m["xt"] = prep["xt"][c]
        m["rec"] = prep["rec"][c]
        maps.append(m)
    return maps


def _assemble(results, prep):
    out = np.zeros((N, D + 4), np.float32)
    for c, r in enumerate(results):
        ids = prep["cand_ids"][c]
        k = len(ids)
        o67 = r["o67"][:, :k]
        out[ids, 0:3] = o67[D:D + 3].T
        out[ids, 3:3 + D] = o67[:D].T
        out[ids, 3 + D] = r["op"][0, :k]
    return out


def kernel(**inputs):
    from concourse.bass_utils import run_bass_kernel_spmd
    prep = _host_prep(inputs["node_features"], inputs["node_operations"],
                      inputs["edge_index"])
    nc = _build(prep)
    maps = _in_maps(prep, inputs["W1"], inputs["b1"], inputs["W2"],
                    inputs["b2"], inputs["W3"], inputs["b3"], inputs["P1"],
                    inputs["pb1"], inputs["P2"], inputs["pb2"])
    res = run_bass_kernel_spmd(nc, maps, core_ids=list(range(CORES)))
    return _assemble(res.results, prep)
